# revision 1
# baseline (speedup 1.0000x reference)
import sys

sys.path.insert(0, "/opt/trn_rl_repo")

import numpy as np

# Problem constants (hardcoded per contract)
B, L, C, K = 8, 16384, 64, 7
T = (L - 2 * K) // 2 + 1  # 8186
HALF = 4096               # t's per half (half-1 ragged: 8186-4096=4090, padded)
TC = 512                  # t-chunk
NCH = HALF // TC          # 8 chunks
WX = 4104                 # column width of folded x tensors (HALF + 8 pad)
LN_EPS = 1e-6

_CACHE = {}


def _build(prelu_slope: float, need_lnsb: bool, need_cb: bool):
    import concourse.bacc as bacc
    import concourse.mybir as mybir
    import concourse.tile as tile

    f32 = mybir.dt.float32
    f16 = mybir.dt.float16
    AF = mybir.ActivationFunctionType
    OP = mybir.AluOpType

    nc = bacc.Bacc("TRN2", target_bir_lowering=False, debug=False, num_devices=8)

    # ---- DRAM parameters (per-core shard data) ----
    dXE = nc.declare_dram_parameter("xe", [128, WX], f16, isOutput=False)
    dXE1 = nc.declare_dram_parameter("xe1", [128, WX], f16, isOutput=False)
    dXO = nc.declare_dram_parameter("xo", [128, WX], f16, isOutput=False)
    dXO1 = nc.declare_dram_parameter("xo1", [128, WX], f16, isOutput=False)
    dWT = nc.declare_dram_parameter("wt", [128, 64 * K], f16, isOutput=False)
    dID = nc.declare_dram_parameter("ident", [128, 128], f16, isOutput=False)
    dON = nc.declare_dram_parameter("ones64", [128, 64], f16, isOutput=False)
    dCK = nc.declare_dram_parameter("ck", [128, 64], f16, isOutput=False)
    dCST = nc.declare_dram_parameter("csts", [128, 4], f32, isOutput=False)
    dOUT = nc.declare_dram_parameter("out", [T, C], f32, isOutput=True)

    from contextlib import ExitStack

    with ExitStack() as es:
        tc = es.enter_context(tile.TileContext(nc))
        cp = es.enter_context(tc.tile_pool(name="const", bufs=1))
        gp = es.enter_context(tc.tile_pool(name="gps", bufs=2, space="PSUM"))
        yp = es.enter_context(tc.tile_pool(name="yps", bufs=1, space="PSUM"))
        zp = es.enter_context(tc.tile_pool(name="zps", bufs=1, space="PSUM"))
        sp = es.enter_context(tc.tile_pool(name="sps", bufs=1, space="PSUM"))
        hp = es.enter_context(tc.tile_pool(name="hsb", bufs=10))
        pp = es.enter_context(tc.tile_pool(name="prod", bufs=16))
        ypool = es.enter_context(tc.tile_pool(name="ysb", bufs=3))
        st1 = es.enter_context(tc.tile_pool(name="st1", bufs=3))
        st2 = es.enter_context(tc.tile_pool(name="st2", bufs=3))
        st3 = es.enter_context(tc.tile_pool(name="st3", bufs=3))
        st4 = es.enter_context(tc.tile_pool(name="st4", bufs=3))
        st5 = es.enter_context(tc.tile_pool(name="st5", bufs=3))
        ynp = es.enter_context(tc.tile_pool(name="ynp", bufs=3))
        pzp = es.enter_context(tc.tile_pool(name="pzp", bufs=3))
        trp = es.enter_context(tc.tile_pool(name="trp", bufs=6))
        op_ = es.enter_context(tc.tile_pool(name="outp", bufs=4))
        if True:
            # ---- load constants ----
            XE = cp.tile([128, WX], f16)
            XE1 = cp.tile([128, WX], f16)
            XO = cp.tile([128, WX], f16)
            XO1 = cp.tile([128, WX], f16)
            WT = cp.tile([128, 64 * K], f16)
            ID = cp.tile([128, 128], f16)
            ON = cp.tile([128, 64], f16)
            CKt = cp.tile([128, 64], f16)
            CST = cp.tile([128, 4], f32)
            for t_, d_ in ((XE, dXE), (XE1, dXE1), (XO, dXO), (XO1, dXO1),
                           (WT, dWT), (ID, dID), (ON, dON), (CKt, dCK), (CST, dCST)):
                nc.sync.dma_start(t_[:], d_[:])

            for i in range(NCH):
                t0 = TC * i
                # ---- G matmuls + tanh: 7 m-planes, each (Ge|Go) (128,1024) ----
                hts = []
                for m in range(K):
                    g = gp.tile([128, 1024], f32)
                    for ci, src_ in ((0, XE), (512, XO)):
                        for h in (0, 1):
                            p0 = 64 * h
                            nc.tensor.matmul(
                                g[p0:p0 + 64, ci:ci + TC],
                                lhsT=WT[p0:p0 + 64, 64 * m:64 * m + 64],
                                rhs=src_[p0:p0 + 64, t0 + 6:t0 + 6 + TC],
                                start=True, stop=True,
                            )
                    ht = hp.tile([128, 1024], f16)
                    nc.scalar.activation(ht[:], g[:], AF.Tanh)
                    hts.append(ht)

                # ---- gating products (14 planes) ----
                prods = []
                for m in range(K):
                    for ci, (xa, xs) in ((0, (XE, XE1)), (512, (XO, XO1))):
                        pr = pp.tile([128, TC], f16)
                        if m % 2 == 0:
                            xap = xa[:, t0 + m:t0 + m + TC]
                        else:
                            xap = xs[:, t0 + m - 1:t0 + m - 1 + TC]
                        nc.vector.tensor_mul(pr[:], xap, hts[m][:, ci:ci + TC])
                        prods.append(pr)

                # ---- accumulate 14 products + skip via identity matmuls ----
                y = yp.tile([128, TC], f32)
                for j, pr in enumerate(prods):
                    nc.tensor.matmul(y[:], lhsT=ID[:], rhs=pr[:],
                                     start=(j == 0), stop=False)
                nc.tensor.matmul(y[:], lhsT=ID[:],
                                 rhs=XE[:, t0 + 6:t0 + 6 + TC],
                                 start=False, stop=True)

                # ---- drain y, square ----
                ysb = ypool.tile([128, TC], f16)
                nc.scalar.copy(ysb[:], y[:])
                ysq = pp.tile([128, TC], f16)
                nc.vector.tensor_mul(ysq[:], ysb[:], ysb[:])

                # ---- LN stats: mean & mean-of-squares via ones-matmul ----
                st = sp.tile([128, 1024], f32)
                for h in (0, 1):
                    p0 = 64 * h
                    nc.tensor.matmul(st[p0:p0 + 64, 0:TC],
                                     lhsT=ON[p0:p0 + 64, :],
                                     rhs=ysb[p0:p0 + 64, :], start=True, stop=True)
                    nc.tensor.matmul(st[p0:p0 + 64, 512:512 + TC],
                                     lhsT=ON[p0:p0 + 64, :],
                                     rhs=ysq[p0:p0 + 64, :], start=True, stop=True)
                mu = st[:, 0:TC]
                m2 = st[:, 512:512 + TC]

                musq = st1.tile([128, TC], f32)
                nc.scalar.activation(musq[:], mu, AF.Square)
                var = st2.tile([128, TC], f32)
                nc.vector.tensor_sub(var[:], m2, musq[:])
                std = st3.tile([128, TC], f32)
                nc.scalar.activation(std[:], var[:], AF.Sqrt, bias=CST[:, 3:4])
                rstd = st4.tile([128, TC], f32)
                scr = st5.tile([128, TC], f32)
                nc.vector.reciprocal_approx_accurate(rstd[:], std[:], scr[:])

                # ---- yn = (y - mu) * rstd  (* s + b) ----
                yc = st1.tile([128, TC], f32)
                nc.vector.tensor_sub(yc[:], ysb[:], mu)
                yn = ynp.tile([128, TC], f16)
                nc.vector.tensor_mul(yn[:], yc[:], rstd[:])
                if need_lnsb:
                    yn2 = ynp.tile([128, TC], f16)
                    nc.vector.tensor_scalar(yn2[:], yn[:], CST[:, 0:1], CST[:, 1:2],
                                            op0=OP.mult, op1=OP.add)
                    yn = yn2

                # ---- 1x1 conv ----
                z = zp.tile([128, TC], f32)
                for h in (0, 1):
                    p0 = 64 * h
                    nc.tensor.matmul(z[p0:p0 + 64, :], lhsT=CKt[p0:p0 + 64, :],
                                     rhs=yn[p0:p0 + 64, :], start=True, stop=True)
                if need_cb:
                    z2 = st2.tile([128, TC], f32)
                    nc.vector.tensor_scalar(z2[:], z[:], CST[:, 2:3], None, op0=OP.add)
                    zsrc = z2
                else:
                    zsrc = z
                # prelu: max(z, slope*z)
                pz = pzp.tile([128, TC], f16)
                nc.scalar.activation(pz[:], zsrc[:], AF.Prelu,
                                     alpha=float(prelu_slope))

                # ---- transpose yn, pz to t-layout; add; store ----
                for h in (0, 1):
                    p0 = 64 * h
                    tb = HALF * h + t0
                    ynT = trp.tile([128, 4, 64], f16)
                    nc.sync.dma_start_transpose(ynT[:], yn[p0:p0 + 64, :])
                    pzT = trp.tile([128, 4, 64], f16)
                    nc.sync.dma_start_transpose(pzT[:], pz[p0:p0 + 64, :])
                    of = op_.tile([128, 4, 64], f32)
                    nc.vector.tensor_add(of[:], ynT[:], pzT[:])
                    if tb + TC <= T:
                        dst = dOUT[tb:tb + TC, :].rearrange(
                            "(j p) c -> p j c", p=128)
                        nc.sync.dma_start(dst, of[:])
                    else:
                        nfull = (T - tb) // 128
                        rem = (T - tb) - nfull * 128
                        if nfull > 0:
                            dst = dOUT[tb:tb + nfull * 128, :].rearrange(
                                "(j p) c -> p j c", p=128)
                            nc.sync.dma_start(dst, of[:, 0:nfull, :])
                        if rem > 0:
                            dst = dOUT[tb + nfull * 128:T, :]
                            nc.sync.dma_start(dst, of[0:rem, nfull, :])

    nc.compile()
    return nc


def _prep_inputs(x, weights, ln_scale, ln_bias, conv_kernel, conv_bias):
    """Host-side prep: returns (per-core input maps, shared consts)."""
    xf = np.asarray(x, dtype=np.float32)
    # shared consts
    WT = np.zeros((128, 64 * K), np.float16)
    for m in range(K):
        wmT = np.asarray(weights[:, :, m]).T.astype(np.float16)  # (c_in, d)
        WT[0:64, 64 * m:64 * m + 64] = wmT
        WT[64:128, 64 * m:64 * m + 64] = wmT
    ID = np.eye(128, dtype=np.float16)
    ON = np.full((128, 64), 1.0 / 64, np.float16)
    CK = np.zeros((128, 64), np.float16)
    ckc = np.asarray(conv_kernel).astype(np.float16)  # (c, o), lhsT layout
    CK[0:64] = ckc
    CK[64:128] = ckc
    CST = np.zeros((128, 4), np.float32)
    s = np.asarray(ln_scale, np.float32)
    b = np.asarray(ln_bias, np.float32)
    cb = np.asarray(conv_bias, np.float32)
    CST[0:64, 0] = s
    CST[64:128, 0] = s
    CST[0:64, 1] = b
    CST[64:128, 1] = b
    CST[0:64, 2] = cb
    CST[64:128, 2] = cb
    CST[:, 3] = LN_EPS

    def fold(a):  # a: (64, 8192) -> (128, WX)
        out = np.zeros((128, WX), np.float16)
        out[0:64, :] = a[:, 0:WX]
        out[64:128, 0:8192 - HALF] = a[:, HALF:8192]
        return out

    in_maps = []
    for bi in range(B):
        xb = xf[bi]                      # (L, C)
        xeT = np.ascontiguousarray(xb[0::2].T).astype(np.float16)  # (64, 8192)
        xoT = np.ascontiguousarray(xb[1::2].T).astype(np.float16)
        xeT1 = np.concatenate([xeT[:, 1:], np.zeros((64, 1), np.float16)], axis=1)
        xoT1 = np.concatenate([xoT[:, 1:], np.zeros((64, 1), np.float16)], axis=1)
        in_maps.append({
            "xe": fold(xeT), "xe1": fold(xeT1),
            "xo": fold(xoT), "xo1": fold(xoT1),
            "wt": WT, "ident": ID, "ones64": ON, "ck": CK, "csts": CST,
        })
    return in_maps


def kernel(x, weights, ln_scale, ln_bias, conv_kernel, conv_bias, prelu_slope):
    from concourse.bass_utils import run_bass_kernel_spmd

    slope = float(np.asarray(prelu_slope))
    need_lnsb = not (np.allclose(np.asarray(ln_scale), 1.0)
                     and np.allclose(np.asarray(ln_bias), 0.0))
    need_cb = not np.allclose(np.asarray(conv_bias), 0.0)

    key = (slope, need_lnsb, need_cb)
    if key not in _CACHE:
        _CACHE[key] = _build(slope, need_lnsb, need_cb)
    nc = _CACHE[key]

    in_maps = _prep_inputs(x, weights, ln_scale, ln_bias, conv_kernel, conv_bias)
    res = run_bass_kernel_spmd(nc, in_maps, core_ids=list(range(8)))
    out = np.stack([res.results[i]["out"] for i in range(B)], axis=0)
    return out.astype(np.float32)



# revision 6
# speedup vs baseline: 3.9431x; 3.9431x over previous
import sys

sys.path.insert(0, "/opt/trn_rl_repo")

import os

os.environ.setdefault("JAX_PLATFORMS", "")

import numpy as np
import jax

# Persistent XLA compilation cache: run_bass_via_pjrt re-creates its jit
# wrapper every call, so without this each call re-runs the full client-side
# neuronx compile (~0.4-0.6s). With the cache, warm calls skip it entirely.
jax.config.update("jax_compilation_cache_dir", "/tmp/jax_pcc")
jax.config.update("jax_persistent_cache_min_compile_time_secs", 0.0)
jax.config.update("jax_persistent_cache_min_entry_size_bytes", -1)

# Problem constants (hardcoded per contract)
B, L, C, K = 8, 16384, 64, 7
T = (L - 2 * K) // 2 + 1  # 8186
HALF = 4096               # t's per half (half-1 ragged: 8186-4096=4090, padded)
TC = 512                  # t-chunk
NCH = HALF // TC          # 8 chunks
WX = 4104                 # column width of folded x tensors (HALF + 8 pad)
NB = (L // 2) // 128      # 64 transpose blocks of 128 t's each
LN_EPS = 1e-6

_CACHE = {}
_SIM_SAFE = False  # CoreSim lacks Prelu; tests substitute Relu via this knob


def _build(prelu_slope: float, need_lnsb: bool, need_cb: bool):
    import concourse.bacc as bacc
    import concourse.mybir as mybir
    import concourse.tile as tile

    f32 = mybir.dt.float32
    f16 = mybir.dt.float16
    AF = mybir.ActivationFunctionType
    OP = mybir.AluOpType

    nc = bacc.Bacc("TRN2", target_bir_lowering=False, debug=False, num_devices=8)

    # ---- DRAM parameters (per-core shard data) ----
    # x is the raw batch element: (16384, 64) f16 viewed as (8192, 128) so
    # each row holds [x[2i, :], x[2i+1, :]] — even/odd de-interleave happens
    # on-device via TensorE transposes.
    dX = nc.declare_dram_parameter("x", [L // 2, 2 * C], f16, isOutput=False)
    dWT = nc.declare_dram_parameter("wt", [128, 64 * K], f16, isOutput=False)
    dID = nc.declare_dram_parameter("ident", [128, 128], f16, isOutput=False)
    dON = nc.declare_dram_parameter("ones64", [128, 64], f16, isOutput=False)
    dCK = nc.declare_dram_parameter("ck", [128, 64], f16, isOutput=False)
    dCST = nc.declare_dram_parameter("csts", [128, 4], f32, isOutput=False)
    dOUT = nc.declare_dram_parameter("out", [T, C], f16, isOutput=True)

    from contextlib import ExitStack

    with ExitStack() as es:
        tc = es.enter_context(tile.TileContext(nc))
        cp = es.enter_context(tc.tile_pool(name="const", bufs=1))
        gp = es.enter_context(tc.tile_pool(name="gps", bufs=4, space="PSUM"))
        yp = es.enter_context(tc.tile_pool(name="yps", bufs=1, space="PSUM"))
        zp = es.enter_context(tc.tile_pool(name="zps", bufs=1, space="PSUM"))
        sp = es.enter_context(tc.tile_pool(name="sps", bufs=1, space="PSUM"))
        hp = es.enter_context(tc.tile_pool(name="hsb", bufs=10))
        pp = es.enter_context(tc.tile_pool(name="prod", bufs=16))
        ypool = es.enter_context(tc.tile_pool(name="ysb", bufs=3))
        st1 = es.enter_context(tc.tile_pool(name="st1", bufs=3))
        st2 = es.enter_context(tc.tile_pool(name="st2", bufs=3))
        st3 = es.enter_context(tc.tile_pool(name="st3", bufs=3))
        st4 = es.enter_context(tc.tile_pool(name="st4", bufs=3))
        st5 = es.enter_context(tc.tile_pool(name="st5", bufs=3))
        ynp = es.enter_context(tc.tile_pool(name="ynp", bufs=3))
        pzp = es.enter_context(tc.tile_pool(name="pzp", bufs=3))
        ofp = es.enter_context(tc.tile_pool(name="ofp", bufs=3))
        trp = es.enter_context(tc.tile_pool(name="trp", bufs=6))
        if True:
            # ---- load constants ----
            WT = cp.tile([128, 64 * K], f16)
            ID = cp.tile([128, 128], f16)
            ON = cp.tile([128, 64], f16)
            CKt = cp.tile([128, 64], f16)
            CST = cp.tile([128, 4], f32)
            for t_, d_ in ((WT, dWT), (ID, dID), (ON, dON), (CKt, dCK), (CST, dCST)):
                nc.sync.dma_start(t_[:], d_[:])

            # ---- phase A: unpack x -> XE/XO (folded layout) on device ----
            XB = cp.tile([128, NB, 128], f16)
            nc.sync.dma_start(XB[:], dX[:].rearrange("(j p) q -> p j q", p=128))

            XE = cp.tile([128, WX], f16)
            XO = cp.tile([128, WX], f16)
            XE1 = cp.tile([128, WX], f16)
            XO1 = cp.tile([128, WX], f16)
            # zero the ragged-tail pad columns of the lower half
            nc.vector.memset(XE[64:128, HALF:WX], 0.0)
            nc.vector.memset(XO[64:128, HALF:WX], 0.0)
            for jg in range(NB // 4):
                tp = gp.tile([128, 512], f32, name="g")
                for q in range(4):
                    j = 4 * jg + q
                    # tp block = XB[:, j, :]^T : rows 0:64 = xe, 64:128 = xo
                    nc.tensor.matmul(tp[:, 128 * q:128 * q + 128],
                                     lhsT=XB[:, j, :], rhs=ID[:],
                                     start=True, stop=True)
                if jg < NB // 8:
                    p0, c0 = 0, 512 * jg
                else:
                    p0, c0 = 64, 512 * (jg - NB // 8)
                nc.scalar.copy(XE[p0:p0 + 64, c0:c0 + 512], tp[0:64, :])
                nc.scalar.copy(XO[p0:p0 + 64, c0:c0 + 512], tp[64:128, :])
                if jg == NB // 8:
                    # upper half's 8 overlap columns (t 4096..4103)
                    nc.scalar.copy(XE[0:64, HALF:WX], tp[0:64, 0:WX - HALF])
                    nc.scalar.copy(XO[0:64, HALF:WX], tp[64:128, 0:WX - HALF])
            # shifted copies (keep vector-op column offsets even)
            nc.sync.dma_start(XE1[:, 0:WX - 1], XE[:, 1:WX])
            nc.sync.dma_start(XO1[:, 0:WX - 1], XO[:, 1:WX])

            for i in range(NCH):
                t0 = TC * i
                # ---- G matmuls + tanh: 7 m-planes, each (Ge|Go) (128,1024) ----
                hts = []
                for m in range(K):
                    ht = hp.tile([128, 1024], f16)
                    for ci, src_ in ((0, XE), (512, XO)):
                        g = gp.tile([128, TC], f32)
                        for h in (0, 1):
                            p0 = 64 * h
                            nc.tensor.matmul(
                                g[p0:p0 + 64, :],
                                lhsT=WT[p0:p0 + 64, 64 * m:64 * m + 64],
                                rhs=src_[p0:p0 + 64, t0 + 6:t0 + 6 + TC],
                                start=True, stop=True,
                            )
                        nc.scalar.activation(ht[:, ci:ci + TC], g[:], AF.Tanh)
                    hts.append(ht)

                # ---- gating products (14 planes) ----
                prods = []
                for m in range(K):
                    for ci, (xa, xs) in ((0, (XE, XE1)), (512, (XO, XO1))):
                        pr = pp.tile([128, TC], f16)
                        if m % 2 == 0:
                            xap = xa[:, t0 + m:t0 + m + TC]
                        else:
                            xap = xs[:, t0 + m - 1:t0 + m - 1 + TC]
                        nc.vector.tensor_mul(pr[:], xap, hts[m][:, ci:ci + TC])
                        prods.append(pr)

                # ---- accumulate 14 products + skip via identity matmuls ----
                y = yp.tile([128, TC], f32)
                for j, pr in enumerate(prods):
                    nc.tensor.matmul(y[:], lhsT=ID[:], rhs=pr[:],
                                     start=(j == 0), stop=False)
                nc.tensor.matmul(y[:], lhsT=ID[:],
                                 rhs=XE[:, t0 + 6:t0 + 6 + TC],
                                 start=False, stop=True)

                # ---- drain y, square ----
                ysb = ypool.tile([128, TC], f16)
                nc.scalar.copy(ysb[:], y[:])
                ysq = pp.tile([128, TC], f16)
                nc.vector.tensor_mul(ysq[:], ysb[:], ysb[:])

                # ---- LN stats: mean & mean-of-squares via ones-matmul ----
                st = sp.tile([128, 1024], f32)
                for h in (0, 1):
                    p0 = 64 * h
                    nc.tensor.matmul(st[p0:p0 + 64, 0:TC],
                                     lhsT=ON[p0:p0 + 64, :],
                                     rhs=ysb[p0:p0 + 64, :], start=True, stop=True)
                    nc.tensor.matmul(st[p0:p0 + 64, 512:512 + TC],
                                     lhsT=ON[p0:p0 + 64, :],
                                     rhs=ysq[p0:p0 + 64, :], start=True, stop=True)
                mu = st[:, 0:TC]
                m2 = st[:, 512:512 + TC]

                musq = st1.tile([128, TC], f32)
                nc.scalar.activation(musq[:], mu, AF.Square)
                var = st2.tile([128, TC], f32)
                nc.vector.tensor_sub(var[:], m2, musq[:])
                std = st3.tile([128, TC], f32)
                nc.scalar.activation(std[:], var[:], AF.Sqrt, bias=CST[:, 3:4])
                rstd = st4.tile([128, TC], f32)
                scr = st5.tile([128, TC], f32)
                nc.vector.reciprocal_approx_accurate(rstd[:], std[:], scr[:])

                # ---- yn = (y - mu) * rstd  (* s + b) ----
                yc = st1.tile([128, TC], f32)
                nc.vector.tensor_sub(yc[:], ysb[:], mu)
                yn = ynp.tile([128, TC], f16)
                nc.vector.tensor_mul(yn[:], yc[:], rstd[:])
                if need_lnsb:
                    yn2 = ynp.tile([128, TC], f16)
                    nc.vector.tensor_scalar(yn2[:], yn[:], CST[:, 0:1], CST[:, 1:2],
                                            op0=OP.mult, op1=OP.add)
                    yn = yn2

                # ---- 1x1 conv ----
                z = zp.tile([128, TC], f32)
                for h in (0, 1):
                    p0 = 64 * h
                    nc.tensor.matmul(z[p0:p0 + 64, :], lhsT=CKt[p0:p0 + 64, :],
                                     rhs=yn[p0:p0 + 64, :], start=True, stop=True)
                if need_cb:
                    z2 = st2.tile([128, TC], f32)
                    nc.vector.tensor_scalar(z2[:], z[:], CST[:, 2:3], None, op0=OP.add)
                    zsrc = z2
                else:
                    zsrc = z
                # prelu: max(z, slope*z)
                pz = pzp.tile([128, TC], f16)
                nc.scalar.activation(pz[:], zsrc[:],
                                     AF.Relu if _SIM_SAFE else AF.Prelu,
                                     alpha=1.0 if _SIM_SAFE else float(prelu_slope))

                # ---- out = yn + pz; transpose to t-layout; store f16 ----
                of = ofp.tile([128, TC], f16)
                nc.vector.tensor_add(of[:], yn[:], pz[:])
                for h in (0, 1):
                    p0 = 64 * h
                    tb = HALF * h + t0
                    ofT = trp.tile([128, 4, 64], f16)
                    nc.sync.dma_start_transpose(ofT[:], of[p0:p0 + 64, :])
                    if tb + TC <= T:
                        dst = dOUT[tb:tb + TC, :].rearrange(
                            "(j p) c -> p j c", p=128)
                        nc.sync.dma_start(dst, ofT[:])
                    else:
                        nfull = (T - tb) // 128
                        rem = (T - tb) - nfull * 128
                        if nfull > 0:
                            dst = dOUT[tb:tb + nfull * 128, :].rearrange(
                                "(j p) c -> p j c", p=128)
                            nc.sync.dma_start(dst, ofT[:, 0:nfull, :])
                        if rem > 0:
                            dst = dOUT[tb + nfull * 128:T, :]
                            nc.sync.dma_start(dst, ofT[0:rem, nfull, :])

    nc.compile()
    return nc


def _prep_inputs(x, weights, ln_scale, ln_bias, conv_kernel, conv_bias):
    """Host-side prep: per-core input maps (raw f16 x + shared consts)."""
    x16 = np.asarray(x, dtype=np.float32).astype(np.float16)  # (B, L, C)
    WT = np.zeros((128, 64 * K), np.float16)
    for m in range(K):
        wmT = np.asarray(weights[:, :, m]).T.astype(np.float16)  # (c_in, d)
        WT[0:64, 64 * m:64 * m + 64] = wmT
        WT[64:128, 64 * m:64 * m + 64] = wmT
    ID = np.eye(128, dtype=np.float16)
    ON = np.full((128, 64), 1.0 / 64, np.float16)
    CK = np.zeros((128, 64), np.float16)
    ckc = np.asarray(conv_kernel).astype(np.float16)  # (c, o), lhsT layout
    CK[0:64] = ckc
    CK[64:128] = ckc
    CST = np.zeros((128, 4), np.float32)
    s = np.asarray(ln_scale, np.float32)
    b = np.asarray(ln_bias, np.float32)
    cb = np.asarray(conv_bias, np.float32)
    CST[0:64, 0] = s
    CST[64:128, 0] = s
    CST[0:64, 1] = b
    CST[64:128, 1] = b
    CST[0:64, 2] = cb
    CST[64:128, 2] = cb
    CST[:, 3] = LN_EPS

    in_maps = []
    for bi in range(B):
        in_maps.append({
            "x": x16[bi].reshape(L // 2, 2 * C),
            "wt": WT, "ident": ID, "ones64": ON, "ck": CK, "csts": CST,
        })
    return in_maps


def kernel(x, weights, ln_scale, ln_bias, conv_kernel, conv_bias, prelu_slope):
    from concourse.bass_utils import run_bass_kernel_spmd

    slope = float(np.asarray(prelu_slope))
    need_lnsb = not (np.allclose(np.asarray(ln_scale), 1.0)
                     and np.allclose(np.asarray(ln_bias), 0.0))
    need_cb = not np.allclose(np.asarray(conv_bias), 0.0)

    key = (slope, need_lnsb, need_cb)
    if key not in _CACHE:
        _CACHE[key] = _build(slope, need_lnsb, need_cb)
    nc = _CACHE[key]

    in_maps = _prep_inputs(x, weights, ln_scale, ln_bias, conv_kernel, conv_bias)
    res = run_bass_kernel_spmd(nc, in_maps, core_ids=list(range(8)))
    out = np.stack([res.results[i]["out"] for i in range(B)], axis=0)
    return out.astype(np.float32)
